# revision 5
# baseline (speedup 1.0000x reference)
"""Distributed causal multi-head attention for TRN2 (8 NeuronCores).

Problem: B=2, T=2048, D=1024, H=16 heads (head_dim 64), causal MHA:
  q,k,v = x@W{q,k,v}+b, q *= dh**-0.5, o = softmax(mask(q k^T)) v, out = o@Wp + bp

Sharding: 2 batch groups x 4 cores (tensor parallel over heads).
Core r handles batch r//4, heads 4*(r%4)..4*(r%4)+3, and output columns
256*(r%4)..  Per core:
  - QKV projections in fp16 on TensorE (q/k produced transposed [hd, t],
    v produced natural [t, hd] with an appended ones-column for softmax sums)
  - scores computed transposed [keys, q] (K=64 contraction, two heads packed
    in the 128x128 PE array via row tiling); exp on ScalarE; causal handled
    by tile skipping + one 128x128 diagonal mask add
  - AV uses exp-weights as the stationary operand -> o natural [q, hd] with
    per-partition row-sums for free (ones column of v); normalize with
    per-partition reciprocal (no cross-partition broadcasts anywhere)
  - o (fp16) AllGathered across the 4-core group in 4 T-chunks of 512,
    overlapped with attention of later chunks
  - final projection reads the gathered o via XBAR DMA-transpose loads and
    computes a 256-column slice of the output per core.
Host side only shards/converts inputs, concatenates outputs, and adds the
bias terms that are mathematically output-constant (bv@Wp + bp; bk cancels
in softmax; bq is applied on device).
"""

import os
import numpy as np

B, T, D, H = 2, 2048, 1024, 16
DH = 64
NCORES = 8
G = 4                  # cores per batch group
HPC = H // G           # heads per core = 4
CD = HPC * DH          # per-core head-dim / out columns = 256
P = 128
NCH = 4                # T chunks for the AllGather pipeline
CHUNK = T // NCH       # 512
KT = T // P            # 16 key tiles
KD = D // P            # 8 contraction tiles for the projections
NEG = -30000.0

_CACHE = {}

# Results of the last device run (for test harnesses): BassKernelResults
LAST_RESULT = None


def _build_nc():
    import concourse.bass as bass
    import concourse.mybir as mybir
    import concourse.tile as tile
    from concourse import bacc
    from contextlib import ExitStack

    fp = mybir.dt.float16
    f32 = mybir.dt.float32
    AF = mybir.ActivationFunctionType

    nc = bacc.Bacc("TRN2", target_bir_lowering=False, debug=False,
                   num_devices=NCORES)

    xT = nc.dram_tensor("xT", [D, T], fp, kind="ExternalInput").ap()
    wq = nc.dram_tensor("wq", [D, CD], fp, kind="ExternalInput").ap()
    wk = nc.dram_tensor("wk", [D, CD], fp, kind="ExternalInput").ap()
    wv = nc.dram_tensor("wv", [D, CD], fp, kind="ExternalInput").ap()
    wp = nc.dram_tensor("wp", [D, CD], fp, kind="ExternalInput").ap()
    bqp = nc.dram_tensor("bqp", [P, 2], f32, kind="ExternalInput").ap()
    maskf = nc.dram_tensor("maskf", [P, P], f32, kind="ExternalInput").ap()
    out = nc.dram_tensor("out", [T, CD], f32, kind="ExternalOutput").ap()

    obounce = nc.dram_tensor("obounce", [NCH, CHUNK, CD], fp).ap()
    gath = nc.dram_tensor("gath", [NCH, G, CHUNK, CD], fp).ap()

    RG = [[0, 1, 2, 3], [4, 5, 6, 7]]

    with tile.TileContext(nc, num_cores=NCORES) as tc, ExitStack() as ctx:
        const = ctx.enter_context(tc.tile_pool(name="const", bufs=1))
        work = ctx.enter_context(tc.tile_pool(name="work", bufs=3))
        expp = ctx.enter_context(tc.tile_pool(name="expp", bufs=36))
        otfp = ctx.enter_context(tc.tile_pool(name="otfp", bufs=2))
        osbp = ctx.enter_context(tc.tile_pool(name="osbp", bufs=8))
        psum = ctx.enter_context(tc.tile_pool(name="psum", bufs=2,
                                              space="PSUM"))

        # ---- persistent SBUF ----
        xT_sb = const.tile([P, KD, T], fp)          # 32 KB/p
        wq_sb = const.tile([P, KD, CD], fp)
        wk_sb = const.tile([P, KD, CD], fp)
        wv_sb = const.tile([P, KD, CD], fp)
        wp_sb = const.tile([P, KD, CD], fp)
        bq_sb = const.tile([P, 2], f32)
        mask_sb = const.tile([P, P], f32)
        qT_sb = const.tile([P, 2, T], fp)           # 2 head-pairs stacked
        kT_sb = const.tile([P, 2, T], fp)
        v_sb = const.tile([P, KT, HPC, DH + 1], fp)  # natural v + ones col

        nc.sync.dma_start(xT_sb[:], xT.rearrange("(k p) t -> p k t", p=P))
        nc.sync.dma_start(wq_sb[:], wq.rearrange("(k p) c -> p k c", p=P))
        nc.sync.dma_start(wk_sb[:], wk.rearrange("(k p) c -> p k c", p=P))
        nc.sync.dma_start(wv_sb[:], wv.rearrange("(k p) c -> p k c", p=P))
        nc.sync.dma_start(wp_sb[:], wp.rearrange("(k p) c -> p k c", p=P))
        nc.sync.dma_start(bq_sb[:], bqp)
        nc.sync.dma_start(mask_sb[:], maskf)
        nc.vector.memset(v_sb[:, :, :, DH:DH + 1], 1.0)

        # ---- phase 1: QKV projections ----
        # q/k transposed [head-dim, t]: lhsT = W tile, rhs = xT
        for m in range(2):                       # head-pair (2 heads x 64)
            for t4 in range(4):                  # t chunks of 512
                psq = psum.tile([P, 512], f32, tag="qkv")
                for k in range(KD):
                    nc.tensor.matmul(
                        psq[:], wq_sb[:, k, m * P:(m + 1) * P],
                        xT_sb[:, k, t4 * 512:(t4 + 1) * 512],
                        start=(k == 0), stop=(k == KD - 1))
                nc.scalar.activation(
                    qT_sb[:, m, t4 * 512:(t4 + 1) * 512], psq[:],
                    AF.Identity, bias=bq_sb[:, m:m + 1])
                psk = psum.tile([P, 512], f32, tag="qkv")
                for k in range(KD):
                    nc.tensor.matmul(
                        psk[:], wk_sb[:, k, m * P:(m + 1) * P],
                        xT_sb[:, k, t4 * 512:(t4 + 1) * 512],
                        start=(k == 0), stop=(k == KD - 1))
                nc.scalar.copy(kT_sb[:, m, t4 * 512:(t4 + 1) * 512], psk[:])
        # v natural [t, hd]: lhsT = xT tile, rhs = W
        for tt in range(KT):
            psv = psum.tile([P, 512], f32, tag="qkv")
            for k in range(KD):
                nc.tensor.matmul(
                    psv[:, :CD], xT_sb[:, k, tt * P:(tt + 1) * P],
                    wv_sb[:, k, :], start=(k == 0), stop=(k == KD - 1))
            nc.vector.tensor_copy(
                out=v_sb[:, tt, :, 0:DH],
                in_=psv[:, :CD].rearrange("p (h d) -> p h d", h=HPC))

        # ---- phase 2: attention per q-chunk + AllGather ----
        exp_tiles = {}
        for c in range(NCH):
            nkt = 4 * (c + 1)                    # causal key tiles this chunk
            osb = [osbp.tile([P, CD], fp, tag="osb", name=f"osb_{c}_{s}")
                   for s in range(4)]
            for pair in range(2):
                for k in range(nkt):
                    for hh in range(2):
                        h = 2 * pair + hh
                        ps_s = psum.tile([P, 512], f32, tag="sc")
                        nc.tensor.matmul(
                            ps_s[:],
                            kT_sb[hh * DH:(hh + 1) * DH, pair,
                                  k * P:(k + 1) * P],
                            qT_sb[hh * DH:(hh + 1) * DH, pair,
                                  c * 512:(c + 1) * 512],
                            start=True, stop=True)
                        e = expp.tile([P, 512], fp, tag="expT")
                        j = k - 4 * c
                        if j >= 0:
                            # diagonal 128x128 block: mask then exp
                            blk = slice(j * P, (j + 1) * P)
                            nc.vector.tensor_add(
                                ps_s[:, blk], ps_s[:, blk], mask_sb[:])
                            nc.scalar.activation(e[:, blk], ps_s[:, blk],
                                                 AF.Exp)
                            if j < 3:
                                rest = slice((j + 1) * P, 512)
                                nc.scalar.activation(e[:, rest],
                                                     ps_s[:, rest], AF.Exp)
                        else:
                            nc.scalar.activation(e[:], ps_s[:], AF.Exp)
                        exp_tiles[(h, k)] = e
                # AV for this pair
                for hh in range(2):
                    h = 2 * pair + hh
                    for s in range(4):
                        nk = 4 * c + s + 1
                        pso = psum.tile([P, DH + 1], f32, tag="o")
                        for k in range(nk):
                            nc.tensor.matmul(
                                pso[:],
                                exp_tiles[(h, k)][:, s * P:(s + 1) * P],
                                v_sb[:, k, h, :],
                                start=(k == 0), stop=(k == nk - 1))
                        rec = work.tile([P, 1], f32, tag="rec")
                        nc.vector.reciprocal(rec[:], pso[:, DH:DH + 1])
                        nc.vector.tensor_scalar_mul(
                            osb[s][:, h * DH:(h + 1) * DH],
                            pso[:, 0:DH], rec[:])
            for s in range(4):
                nc.sync.dma_start(
                    obounce[c, s * P:(s + 1) * P, :], osb[s][:])
            nc.gpsimd.collective_compute(
                "AllGather", bass.mybir.AluOpType.bypass,
                replica_groups=RG,
                ins=[obounce[c]],
                outs=[gath[c]])

        # ---- phase 3: output projection per chunk ----
        for c in range(NCH):
            oTf = otfp.tile([P, KD, CHUNK], fp, tag="oTf")
            for k in range(KD):
                r, lc = k // 2, (k % 2) * P
                nc.sync.dma_start_transpose(
                    oTf[:, k, :], gath[c, r, :, lc:lc + P])
            for s in range(4):
                psp = psum.tile([P, 512], f32, tag="proj")
                for k in range(KD):
                    nc.tensor.matmul(
                        psp[:, :CD], oTf[:, k, s * P:(s + 1) * P],
                        wp_sb[:, k, :], start=(k == 0), stop=(k == KD - 1))
                outsb = work.tile([P, CD], f32, tag="outsb")
                nc.scalar.copy(outsb[:], psp[:, :CD])
                nc.sync.dma_start(
                    out[(c * 4 + s) * P:(c * 4 + s + 1) * P, :], outsb[:])

    nc.finalize()
    return nc


def _get_nc():
    if "nc" not in _CACHE:
        _CACHE["nc"] = _build_nc()
    return _CACHE["nc"]


def kernel(x, Wq, bq, Wk, bk, Wv, bv, Wp, bp):
    global LAST_RESULT
    from concourse.bass_utils import run_bass_kernel_spmd

    x = np.asarray(x, dtype=np.float32)
    Wq = np.asarray(Wq, dtype=np.float32)
    Wk = np.asarray(Wk, dtype=np.float32)
    Wv = np.asarray(Wv, dtype=np.float32)
    Wp = np.asarray(Wp, dtype=np.float32)
    bq = np.asarray(bq, dtype=np.float32)
    bk = np.asarray(bk, dtype=np.float32)
    bv = np.asarray(bv, dtype=np.float32)
    bp = np.asarray(bp, dtype=np.float32)

    s = DH ** -0.5
    maskf = np.where(
        np.arange(P)[:, None] <= np.arange(P)[None, :], 0.0, NEG
    ).astype(np.float32)

    in_maps = []
    for r in range(NCORES):
        b, g = r // G, r % G
        cols = slice(g * CD, (g + 1) * CD)
        bq_cols = (bq[cols] * s).astype(np.float32)
        in_maps.append({
            "xT": np.ascontiguousarray(x[b].T).astype(np.float16),
            "wq": (Wq[:, cols] * s).astype(np.float16),
            "wk": np.ascontiguousarray(Wk[:, cols]).astype(np.float16),
            "wv": np.ascontiguousarray(Wv[:, cols]).astype(np.float16),
            "wp": np.ascontiguousarray(Wp[:, cols]).astype(np.float16),
            "bqp": np.ascontiguousarray(bq_cols.reshape(2, P).T),
            "maskf": maskf,
        })

    nc = _get_nc()
    res = run_bass_kernel_spmd(
        nc, in_maps, core_ids=list(range(NCORES)),
        trace=bool(int(os.environ.get("KERNEL_TRACE", "0"))))
    LAST_RESULT = res

    out = np.empty((B, T, D), dtype=np.float32)
    for r in range(NCORES):
        b, g = r // G, r % G
        out[b, :, g * CD:(g + 1) * CD] = res.results[r]["out"]
    # bias terms that are constant w.r.t. the data path:
    #   v-bias passes through softmax rows (sum=1) -> + bv@Wp; plus bp.
    #   (bk shifts every logit in a row equally -> cancels in softmax.)
    out += (bv @ Wp + bp)[None, None, :]
    return out


# revision 9
# speedup vs baseline: 1.1824x; 1.1824x over previous
"""Distributed causal multi-head attention for TRN2 (8 NeuronCores).

Problem: B=2, T=2048, D=1024, H=16 heads (head_dim 64), causal MHA:
  q,k,v = x@W{q,k,v}+b, q *= dh**-0.5, o = softmax(mask(q k^T)) v, out = o@Wp + bp

Sharding: 2 batch groups x 4 cores (tensor parallel over heads).
Core r handles batch r//4, heads 4*(r%4)..4*(r%4)+3, and output columns
256*(r%4)..  Per core:
  - QKV projections in fp16 on TensorE (q/k produced transposed [hd, t],
    v produced natural [t, hd] with an appended ones-column for softmax sums)
  - scores computed transposed [keys, q] (K=64 contraction, two heads packed
    in the 128x128 PE array via row tiling); exp on ScalarE; causal handled
    by tile skipping + one 128x128 diagonal mask add
  - AV uses exp-weights as the stationary operand -> o natural [q, hd] with
    per-partition row-sums for free (ones column of v); normalize with
    per-partition reciprocal (no cross-partition broadcasts anywhere)
  - o (fp16) AllGathered across the 4-core group in 4 T-chunks of 512,
    overlapped with attention of later chunks
  - final projection reads the gathered o via XBAR DMA-transpose loads and
    computes a 256-column slice of the output per core.
Host side only shards/converts inputs, concatenates outputs, and adds the
bias terms that are mathematically output-constant (bv@Wp + bp; bk cancels
in softmax; bq is applied on device).
"""

import os
import numpy as np

B, T, D, H = 2, 2048, 1024, 16
DH = 64
NCORES = 8
G = 4                  # cores per batch group
HPC = H // G           # heads per core = 4
CD = HPC * DH          # per-core head-dim / out columns = 256
P = 128
NCH = 4                # T chunks for the AllGather pipeline
CHUNK = T // NCH       # 512
KT = T // P            # 16 key tiles
KD = D // P            # 8 contraction tiles for the projections
NEG = -30000.0

_CACHE = {}

# Results of the last device run (for test harnesses): BassKernelResults
LAST_RESULT = None


def _build_nc():
    import concourse.bass as bass
    import concourse.mybir as mybir
    import concourse.tile as tile
    from concourse import bacc
    from contextlib import ExitStack

    fp = mybir.dt.float16
    f32 = mybir.dt.float32
    AF = mybir.ActivationFunctionType

    nc = bacc.Bacc("TRN2", target_bir_lowering=False, debug=False,
                   num_devices=NCORES)

    xT = nc.dram_tensor("xT", [D, T], fp, kind="ExternalInput").ap()
    wq = nc.dram_tensor("wq", [D, CD], fp, kind="ExternalInput").ap()
    wk = nc.dram_tensor("wk", [D, CD], fp, kind="ExternalInput").ap()
    wv = nc.dram_tensor("wv", [D, CD], fp, kind="ExternalInput").ap()
    wp = nc.dram_tensor("wp", [D, CD], fp, kind="ExternalInput").ap()
    bqp = nc.dram_tensor("bqp", [P, 2], f32, kind="ExternalInput").ap()
    maskf = nc.dram_tensor("maskf", [P, P], fp, kind="ExternalInput").ap()
    out = nc.dram_tensor("out", [T, CD], f32, kind="ExternalOutput").ap()

    obounce = nc.dram_tensor("obounce", [NCH, CHUNK, CD], fp).ap()
    gath = nc.dram_tensor("gath", [NCH, G, CHUNK, CD], fp).ap()

    RG = [[0, 1, 2, 3], [4, 5, 6, 7]]

    with tile.TileContext(nc, num_cores=NCORES) as tc, ExitStack() as ctx:
        const = ctx.enter_context(tc.tile_pool(name="const", bufs=1))
        work = ctx.enter_context(tc.tile_pool(name="work", bufs=3))
        expp = ctx.enter_context(tc.tile_pool(name="expp", bufs=20))
        otfp = ctx.enter_context(tc.tile_pool(name="otfp", bufs=2))
        osbp = ctx.enter_context(tc.tile_pool(name="osbp", bufs=8))
        psum = ctx.enter_context(tc.tile_pool(name="psum", bufs=2,
                                              space="PSUM"))

        # ---- persistent SBUF ----
        xT_sb = const.tile([P, KD, T], fp)          # 32 KB/p
        wq_sb = const.tile([P, KD, CD], fp)
        wk_sb = const.tile([P, KD, CD], fp)
        wv_sb = const.tile([P, KD, CD], fp)
        wp_sb = const.tile([P, KD, CD], fp)
        bq_sb = const.tile([P, 2], f32)
        mask_sb = const.tile([P, P], fp)            # 0/1 lower triangle
        qT_sb = const.tile([P, 2, T], fp)           # 2 head-pairs stacked
        kT_sb = const.tile([P, 2, T], fp)
        v_sb = const.tile([P, KT, HPC, DH + 1], fp)  # natural v + ones col

        nc.sync.dma_start(xT_sb[:], xT.rearrange("(k p) t -> p k t", p=P))
        nc.sync.dma_start(wq_sb[:], wq.rearrange("(k p) c -> p k c", p=P))
        nc.sync.dma_start(wk_sb[:], wk.rearrange("(k p) c -> p k c", p=P))
        nc.sync.dma_start(wv_sb[:], wv.rearrange("(k p) c -> p k c", p=P))
        nc.sync.dma_start(wp_sb[:], wp.rearrange("(k p) c -> p k c", p=P))
        nc.sync.dma_start(bq_sb[:], bqp)
        nc.sync.dma_start(mask_sb[:], maskf)
        nc.vector.memset(v_sb[:, :, :, DH:DH + 1], 1.0)

        def qkv_chunk(t4):
            """Project q,k (transposed) and v (natural) for T-chunk t4."""
            for m in range(2):                   # head-pair (2 heads x 64)
                psqk = psum.tile([P, 1024], f32, tag="big",
                                 name=f"psqk_{t4}_{m}")
                for k in range(KD):
                    nc.tensor.matmul(
                        psqk[:, 0:512], wq_sb[:, k, m * P:(m + 1) * P],
                        xT_sb[:, k, t4 * 512:(t4 + 1) * 512],
                        start=(k == 0), stop=(k == KD - 1))
                for k in range(KD):
                    nc.tensor.matmul(
                        psqk[:, 512:1024], wk_sb[:, k, m * P:(m + 1) * P],
                        xT_sb[:, k, t4 * 512:(t4 + 1) * 512],
                        start=(k == 0), stop=(k == KD - 1))
                nc.vector.tensor_scalar_add(
                    qT_sb[:, m, t4 * 512:(t4 + 1) * 512], psqk[:, 0:512],
                    bq_sb[:, m:m + 1])
                nc.vector.tensor_copy(
                    kT_sb[:, m, t4 * 512:(t4 + 1) * 512], psqk[:, 512:1024])
            for tt in range(4 * t4, 4 * t4 + 4):
                psv = psum.tile([P, 256], f32, tag="pv", name=f"psv_{tt}")
                for k in range(KD):
                    nc.tensor.matmul(
                        psv[:, :CD], xT_sb[:, k, tt * P:(tt + 1) * P],
                        wv_sb[:, k, :], start=(k == 0), stop=(k == KD - 1))
                nc.vector.tensor_copy(
                    out=v_sb[:, tt, :, 0:DH],
                    in_=psv[:, :CD].rearrange("p (h d) -> p h d", h=HPC))

        def attention_chunk(c):
            """Causal attention for q-chunk c over key tiles 0..4c+3,
            normalized output -> obounce[c] -> AllGather."""
            nkt = 4 * (c + 1)
            npair = nkt // 2
            exp_tiles = {}
            osb = [osbp.tile([P, CD], fp, tag="osb", name=f"osb_{c}_{s}")
                   for s in range(4)]
            for pair in range(2):
                for hh in range(2):
                    h = 2 * pair + hh
                    lo, hi = hh * DH, (hh + 1) * DH
                    for p in range(npair):
                        ps_s = psum.tile([P, 1024], f32, tag="big",
                                         name=f"ps_s_{c}_{h}_{p}")
                        for half in range(2):
                            k = 2 * p + half
                            nc.tensor.matmul(
                                ps_s[:, half * 512:(half + 1) * 512],
                                kT_sb[lo:hi, pair, k * P:(k + 1) * P],
                                qT_sb[lo:hi, pair,
                                      c * 512:(c + 1) * 512],
                                start=True, stop=True)
                        e = expp.tile([P, 1024], fp, tag="expT",
                                      name=f"expT_{c}_{h}_{p}")
                        nc.scalar.activation(e[:], ps_s[:], AF.Exp)
                        # diagonal pairs: zero the upper triangle of the
                        # two 128x128 diagonal blocks (post-exp 0/1 mult)
                        if p >= 2 * c:
                            for half in range(2):
                                j = 2 * (p - 2 * c) + half
                                col = half * 512 + j * P
                                nc.vector.tensor_mul(
                                    e[:, col:col + P], e[:, col:col + P],
                                    mask_sb[:])
                        exp_tiles[(h, p)] = e
                    # AV for this head
                    for s in range(4):
                        nk = 4 * c + s + 1
                        pso = psum.tile([P, DH + 1], f32, tag="o",
                                        name=f"pso_{c}_{h}_{s}")
                        for k in range(nk):
                            nc.tensor.matmul(
                                pso[:],
                                exp_tiles[(h, k // 2)][
                                    :, (k % 2) * 512 + s * P:
                                    (k % 2) * 512 + (s + 1) * P],
                                v_sb[:, k, h, :],
                                start=(k == 0), stop=(k == nk - 1))
                        rec = work.tile([P, 1], f32, tag="rec",
                                        name=f"rec_{c}_{h}_{s}")
                        nc.vector.reciprocal(rec[:], pso[:, DH:DH + 1])
                        nc.vector.tensor_scalar_mul(
                            osb[s][:, h * DH:(h + 1) * DH],
                            pso[:, 0:DH], rec[:])
            for s in range(4):
                nc.sync.dma_start(
                    obounce[c, s * P:(s + 1) * P, :], osb[s][:])
            nc.gpsimd.collective_compute(
                "AllGather", bass.mybir.AluOpType.bypass,
                replica_groups=RG,
                ins=[obounce[c]],
                outs=[gath[c]])

        def proj_chunk(c):
            """Output projection for T-chunk c from the gathered heads."""
            oTf = otfp.tile([P, KD, CHUNK], fp, tag="oTf",
                            name=f"oTf_{c}")
            for k in range(KD):
                r, lc = k // 2, (k % 2) * P
                nc.sync.dma_start_transpose(
                    oTf[:, k, :], gath[c, r, :, lc:lc + P])
            for s in range(4):
                psp = psum.tile([P, 256], f32, tag="pv",
                                name=f"psp_{c}_{s}")
                for k in range(KD):
                    nc.tensor.matmul(
                        psp[:, :CD], oTf[:, k, s * P:(s + 1) * P],
                        wp_sb[:, k, :], start=(k == 0), stop=(k == KD - 1))
                outsb = work.tile([P, CD], f32, tag="outsb",
                                  name=f"outsb_{c}_{s}")
                nc.scalar.copy(outsb[:], psp[:, :CD])
                nc.sync.dma_start(
                    out[(c * 4 + s) * P:(c * 4 + s + 1) * P, :], outsb[:])

        # pipeline: qkv(c) -> attention(c) [-> AG(c)], proj(c) two
        # iterations later so its matmuls never head-of-line block the PE
        for c in range(NCH):
            qkv_chunk(c)
            if c >= 2:
                proj_chunk(c - 2)
            attention_chunk(c)
        proj_chunk(NCH - 2)
        proj_chunk(NCH - 1)

    nc.finalize()
    return nc


def _get_nc():
    if "nc" not in _CACHE:
        _CACHE["nc"] = _build_nc()
    return _CACHE["nc"]


def kernel(x, Wq, bq, Wk, bk, Wv, bv, Wp, bp):
    global LAST_RESULT
    from concourse.bass_utils import run_bass_kernel_spmd

    x = np.asarray(x, dtype=np.float32)
    Wq = np.asarray(Wq, dtype=np.float32)
    Wk = np.asarray(Wk, dtype=np.float32)
    Wv = np.asarray(Wv, dtype=np.float32)
    Wp = np.asarray(Wp, dtype=np.float32)
    bq = np.asarray(bq, dtype=np.float32)
    bk = np.asarray(bk, dtype=np.float32)
    bv = np.asarray(bv, dtype=np.float32)
    bp = np.asarray(bp, dtype=np.float32)

    s = DH ** -0.5
    maskf = np.where(
        np.arange(P)[:, None] <= np.arange(P)[None, :], 1.0, 0.0
    ).astype(np.float16)

    in_maps = []
    for r in range(NCORES):
        b, g = r // G, r % G
        cols = slice(g * CD, (g + 1) * CD)
        bq_cols = (bq[cols] * s).astype(np.float32)
        in_maps.append({
            "xT": np.ascontiguousarray(x[b].T).astype(np.float16),
            "wq": (Wq[:, cols] * s).astype(np.float16),
            "wk": np.ascontiguousarray(Wk[:, cols]).astype(np.float16),
            "wv": np.ascontiguousarray(Wv[:, cols]).astype(np.float16),
            "wp": np.ascontiguousarray(Wp[:, cols]).astype(np.float16),
            "bqp": np.ascontiguousarray(bq_cols.reshape(2, P).T),
            "maskf": maskf,
        })

    nc = _get_nc()
    res = run_bass_kernel_spmd(
        nc, in_maps, core_ids=list(range(NCORES)),
        trace=bool(int(os.environ.get("KERNEL_TRACE", "0"))))
    LAST_RESULT = res

    out = np.empty((B, T, D), dtype=np.float32)
    for r in range(NCORES):
        b, g = r // G, r % G
        out[b, :, g * CD:(g + 1) * CD] = res.results[r]["out"]
    # bias terms that are constant w.r.t. the data path:
    #   v-bias passes through softmax rows (sum=1) -> + bv@Wp; plus bp.
    #   (bk shifts every logit in a row equally -> cancels in softmax.)
    out += (bv @ Wp + bp)[None, None, :]
    return out
